# revision 1
# baseline (speedup 1.0000x reference)
"""AFSIAttention TRN2 kernel: data-parallel over batch across 8 NeuronCores.

Shards the B=16 batch as 2 samples per core (all compute is per-sample:
DCT features, freq_attn, attention). Each shard runs the fused attention
on its own NeuronCore; results are gathered to the full (B, N, DIM) output.

Self-contained: shapes/sharding hardcoded; DCT basis precomputed (input-
independent constant, like a rotary cos/sin table).
"""

import numpy as np
import jax
import jax.numpy as jnp

B, N, DIM, H = 16, 1024, 512, 8
HEAD_DIM = DIM // H
SCALE = HEAD_DIM ** -0.5
M = 32  # ceil(sqrt(N)); 32*32 == 1024 == N exactly, so no padding needed
N_CORES = 8
B_SHARD = B // N_CORES  # 2 samples per core


def _dct_matrix_np(m: int) -> np.ndarray:
    # Orthonormal DCT-II matrix (matches cv2.dct / reference._dct_matrix).
    n = np.arange(m, dtype=np.float32)
    k = n[:, None]
    D = np.sqrt(2.0 / m) * np.cos(np.pi * (2.0 * n[None, :] + 1.0) * k / (2.0 * m))
    D[0, :] = np.sqrt(1.0 / m)
    return D.astype(np.float32)


_DCT_D = _dct_matrix_np(M)


def _shard_forward(x, qkv_w, qkv_b, proj_w, proj_b, h1_w, h1_b, h2_w, h2_b,
                   freq_w, D):
    """Full AFSI attention for one batch shard x: (B_SHARD, N, DIM)."""
    b, n, c = B_SHARD, N, DIM

    # QKV projection -> q, k, v each (b, H, N, head_dim)
    qkv = (x @ qkv_w + qkv_b).reshape(b, n, 3, H, c // H)
    qkv = jnp.transpose(qkv, (2, 0, 3, 1, 4))
    q, k, v = qkv[0] * SCALE, qkv[1], qkv[2]
    attn = jnp.einsum('bhnd,bhmd->bhnm', q, k)

    # DCT features over the channel-mean signal. N == M*M so the reshape
    # is exact (reference pads to m*m, which is a no-op here).
    x_mean = x.mean(axis=-1)                                # (b, N)
    A = x_mean.reshape(b, M, M)
    dct2 = jnp.einsum('km,bmn,ln->bkl', D, A, D)
    dct_feat = dct2.reshape(b, M * M)[:, :n]
    dct_feat = jnp.clip(dct_feat, -10.0, 10.0)
    norm = jnp.linalg.norm(dct_feat, axis=1, keepdims=True) + 1e-5
    dct_feat = dct_feat / norm                              # (b, N)

    # Per-head frequency weights from the channel-avg MLP.
    x_avg = x.mean(axis=1)                                  # (b, C)
    head_w = jax.nn.relu(x_avg @ h1_w + h1_b) @ h2_w + h2_b  # (b, H)

    # freq_attn is rank-1 per sample: s_b * outer(dct_feat, dct_feat) with
    # s_b = sum_h head_w[b,h]^2 — computed exactly as the reference does.
    s = jnp.sum(head_w * head_w, axis=-1)                   # (b,)
    freq_attn = (s[:, None, None]
                 * dct_feat[:, :, None] * dct_feat[:, None, :])  # (b, N, N)
    row_sum = jnp.maximum(freq_attn.sum(axis=-1, keepdims=True), 1e-5)
    freq_attn = jnp.clip(freq_attn / row_sum, 0.0, 1.0)

    aw = jax.nn.sigmoid(freq_w)
    attn = (1.0 - aw) * attn + aw * freq_attn[:, None, :, :]
    attn = jax.nn.softmax(attn, axis=-1)

    out = jnp.einsum('bhnm,bhmd->bhnd', attn, v)
    out = jnp.transpose(out, (0, 2, 1, 3)).reshape(b, n, c)
    return out @ proj_w + proj_b


_JIT_SHARD = jax.jit(_shard_forward)


def kernel(x, qkv_w, qkv_b, proj_w, proj_b, h1_w, h1_b, h2_w, h2_b, freq_w):
    x = np.asarray(x, dtype=np.float32)
    weights = [np.asarray(a, dtype=np.float32)
               for a in (qkv_w, qkv_b, proj_w, proj_b,
                         h1_w, h1_b, h2_w, h2_b)]
    freq_w = np.asarray(freq_w, dtype=np.float32)

    devs = jax.devices()[:N_CORES]

    # Replicate weights + DCT basis on every core; shard x over batch.
    per_dev_args = []
    for i, dev in enumerate(devs):
        shard = jax.device_put(x[i * B_SHARD:(i + 1) * B_SHARD], dev)
        args = [shard]
        args += [jax.device_put(w, dev) for w in weights]
        args.append(jax.device_put(freq_w, dev))
        args.append(jax.device_put(_DCT_D, dev))
        per_dev_args.append(args)

    # Async dispatch: all 8 cores run their shard concurrently.
    outs = [_JIT_SHARD(*args) for args in per_dev_args]
    out = np.concatenate([np.asarray(o) for o in outs], axis=0)
    return out.astype(np.float32)


# revision 2
# speedup vs baseline: 1.6106x; 1.6106x over previous
"""AFSIAttention TRN2 kernel: data-parallel over batch across 8 NeuronCores.

Shards the B=16 batch as 2 samples per core (all compute is per-sample:
DCT features, freq_attn, attention). Each shard runs the fused attention
on its own NeuronCore; results are gathered to the full (B, N, DIM) output.

Self-contained: shapes/sharding hardcoded; DCT basis precomputed (input-
independent constant, like a rotary cos/sin table).
"""

import numpy as np
import jax
import jax.numpy as jnp

B, N, DIM, H = 16, 1024, 512, 8
HEAD_DIM = DIM // H
SCALE = HEAD_DIM ** -0.5
M = 32  # ceil(sqrt(N)); 32*32 == 1024 == N exactly, so no padding needed
N_CORES = 8
B_SHARD = B // N_CORES  # 2 samples per core


def _dct_matrix_np(m: int) -> np.ndarray:
    # Orthonormal DCT-II matrix (matches cv2.dct / reference._dct_matrix).
    n = np.arange(m, dtype=np.float32)
    k = n[:, None]
    D = np.sqrt(2.0 / m) * np.cos(np.pi * (2.0 * n[None, :] + 1.0) * k / (2.0 * m))
    D[0, :] = np.sqrt(1.0 / m)
    return D.astype(np.float32)


_DCT_D = _dct_matrix_np(M)


def _shard_forward(x, qkv_w, qkv_b, proj_w, proj_b, h1_w, h1_b, h2_w, h2_b,
                   freq_w, D):
    """Full AFSI attention for one batch shard x: (B_SHARD, N, DIM)."""
    b, n, c = B_SHARD, N, DIM

    # QKV projection -> q, k, v each (b, H, N, head_dim)
    qkv = (x @ qkv_w + qkv_b).reshape(b, n, 3, H, c // H)
    qkv = jnp.transpose(qkv, (2, 0, 3, 1, 4))
    q, k, v = qkv[0] * SCALE, qkv[1], qkv[2]
    attn = jnp.einsum('bhnd,bhmd->bhnm', q, k)

    # DCT features over the channel-mean signal. N == M*M so the reshape
    # is exact (reference pads to m*m, which is a no-op here).
    x_mean = x.mean(axis=-1)                                # (b, N)
    A = x_mean.reshape(b, M, M)
    dct2 = jnp.einsum('km,bmn,ln->bkl', D, A, D)
    dct_feat = dct2.reshape(b, M * M)[:, :n]
    dct_feat = jnp.clip(dct_feat, -10.0, 10.0)
    norm = jnp.linalg.norm(dct_feat, axis=1, keepdims=True) + 1e-5
    dct_feat = dct_feat / norm                              # (b, N)

    # Per-head frequency weights from the channel-avg MLP.
    x_avg = x.mean(axis=1)                                  # (b, C)
    head_w = jax.nn.relu(x_avg @ h1_w + h1_b) @ h2_w + h2_b  # (b, H)

    # freq_attn is rank-1 per sample: s_b * outer(dct_feat, dct_feat) with
    # s_b = sum_h head_w[b,h]^2 — computed exactly as the reference does.
    s = jnp.sum(head_w * head_w, axis=-1)                   # (b,)
    freq_attn = (s[:, None, None]
                 * dct_feat[:, :, None] * dct_feat[:, None, :])  # (b, N, N)
    row_sum = jnp.maximum(freq_attn.sum(axis=-1, keepdims=True), 1e-5)
    freq_attn = jnp.clip(freq_attn / row_sum, 0.0, 1.0)

    aw = jax.nn.sigmoid(freq_w)
    attn = (1.0 - aw) * attn + aw * freq_attn[:, None, :, :]
    attn = jax.nn.softmax(attn, axis=-1)

    out = jnp.einsum('bhnm,bhmd->bhnd', attn, v)
    out = jnp.transpose(out, (0, 2, 1, 3)).reshape(b, n, c)
    return out @ proj_w + proj_b


_PFN = None


def _get_pfn():
    global _PFN
    if _PFN is None:
        # One SPMD dispatch: batch axis sharded across the 8 cores,
        # weights + DCT basis broadcast.
        _PFN = jax.pmap(_shard_forward, in_axes=(0,) + (None,) * 10,
                        devices=jax.devices()[:N_CORES])
    return _PFN


def kernel(x, qkv_w, qkv_b, proj_w, proj_b, h1_w, h1_b, h2_w, h2_b, freq_w):
    x = np.asarray(x, dtype=np.float32)
    weights = [np.asarray(a, dtype=np.float32)
               for a in (qkv_w, qkv_b, proj_w, proj_b,
                         h1_w, h1_b, h2_w, h2_b)]
    freq_w = np.asarray(freq_w, dtype=np.float32)

    xs = x.reshape(N_CORES, B_SHARD, N, DIM)
    out = _get_pfn()(xs, *weights, freq_w, _DCT_D)
    return np.asarray(out).reshape(B, N, DIM).astype(np.float32)
